# revision 22
# baseline (speedup 1.0000x reference)
"""CXLoss kernel for trn2 (8 NeuronCores).

Math
----
reference computes, per sample n:
  dot[q,p]   = <Tn[:,q], In[:,p]>          (C=256 contraction, P=4096)
  raw        = (1-dot)/2
  mn[p]      = min_q raw[q,p];  denom = mn + eps
  w[q,p]     = exp((1 - raw/denom)/sigma)
  cx_feat    = w / sum_q w
  cx         = 0.5*cx_feat + 0.5*cx_sp
  loss       = mean_n -log(mean_q max_p cx[q,p])

The spatial branch cx_sp is input-independent and, in fp32, is EXACTLY the
identity matrix: the spatial distance matrix has 0 on the diagonal, so
mn=0, denom=eps=1e-5 and the softmax temperature collapses — every
off-diagonal weight underflows to 0.  Hence
  max_p cx[q,p] = 0.5 + 0.5*cx_feat[q,q]
and only the DIAGONAL of cx_feat (plus the column sums S) is needed:
  loss = mean_n(-log(mean_q(0.5 + 0.5*w[q,q]/S[q])))
(verified vs reference: rel err ~4e-7).

Fold raw/mn into dot:  w = exp(dot*s_p + b_p) with
  denom = (1-mx_p)/2 + eps   (mx_p = max_q dot[q,p])
  s_p = 5/denom,  b_p = 10 - s_p       (sigma=0.1, B=1)

Sharding: 8 cores = 2 samples x 4 slices of the p axis (1024 rows each).
Per core, 8 p-blocks of 128 rows x 4096 q columns. q is complete per core
so mx/S are core-local; no collectives.  Each core's T matrix is rolled by
-p_slice so the diagonal band sits at fixed local columns [pb*128,(pb+1)*128)
(one SPMD program for all cores).

Per block: fp16 matmul -> PSUM; the first H0 q-columns are copied by ACT to
SBUF (staging, frees PSUM without waiting on the row-max chain) and the
remaining columns are re-computed by PE in pass 2 (ACT copy and PE recompute
have identical marginal cost ~0.83ns/col, so the split ratio balances the
two engines); DVE row-max from PSUM; ACT exp with per-partition scale/bias
(scale folds 5/denom, bias folds 10-5/denom AND the affine (1-dot)/2) and
accum_out row-sums; DVE extracts the diagonal band via identity-mask
multiply + row reduce.  The pass-2 exp lags pass-1 by one block so the
s/b chain is off the critical path.
Outputs per core: [128, 16] = (diag per block || S per block); host combines.

Engine budget per core (cost model): PE ~50us, ACT ~47us, DVE ~42us busy,
~66us wall.  The walrus build here allows only ONE sync wait per
instruction; _legalize_waits hoists extras onto EventSemaphore carriers.
"""

import numpy as np

# ---- problem constants (hardcoded; grading env has only this file) ----
N_SAMP, C, H, W = 2, 256, 64, 64
P = H * W                      # 4096
N_CORES = 8
CORES_PER_SAMPLE = 4
PSL = P // CORES_PER_SAMPLE    # 1024 p rows per core
PBLK = 128
NBLK = PSL // PBLK             # 8 blocks per core
KCH = C // 128                 # 2 contraction chunks
H0 = 2048                      # q columns staged via ACT copy (2 psum tiles)
H1 = P - H0                    # q columns recomputed for the exp pass
QT = 1024                      # psum tile width (2 banks)
FD = 512                       # matmul free-dim chunk (1 bank)
EPS = 1e-5

_CACHE = {}


def _legalize_waits(nc, max_waits=1):
    """The pinned walrus rejects instructions with more than one sync wait
    ("Too many sync wait commands").  Hoist excess waits onto standalone
    EventSemaphore carrier instructions on the same engine queue (queue is
    in-order, so a preceding wait is equivalent)."""
    from concourse import mybir

    n = 0
    for fn in nc.m.functions:
        for blk in fn.blocks:
            new_insts = []
            for inst in blk.instructions:
                si = getattr(inst, "sync_info", None)
                waits = list(si.on_wait) if (si is not None and si.on_wait) else []
                if len(waits) > max_waits:
                    excess = waits[:len(waits) - max_waits]
                    si.on_wait = waits[len(waits) - max_waits:]
                    for w in excess:
                        n += 1
                        new_insts.append(mybir.InstEventSemaphore(
                            name=f"{inst.name}-lw{n}",
                            engine=inst.engine,
                            ins=[], outs=[],
                            sync_info=mybir.SyncInfo(on_wait=[w], on_update=[]),
                        ))
                new_insts.append(inst)
            blk.instructions = new_insts
    return n


def _build_nc(H0=1024, QT=QT, ps_bufs=4, stg_bufs=2, w_bufs=2, lag=1,
              max_from_psum=True, last_full=False, rch_plan=(512, 512, 1024, 1024, 1024),
              repeats=1, h0_sched=(2048, 1024, 1024, 1024, 1024, 1024, 1024, 1024)):
    import concourse.bass as bass
    from concourse import mybir
    from concourse.tile import TileContext

    f16 = mybir.dt.float16
    f32 = mybir.dt.float32
    Alu = mybir.AluOpType
    Act = mybir.ActivationFunctionType
    X = mybir.AxisListType.X

    # per-block staged-column counts; optionally the last block is fully
    # staged (no pass-2 recompute) to shorten the kernel tail
    H0s = [H0] * NBLK
    if last_full:
        H0s[NBLK - 1] = P
    if h0_sched is not None:
        H0s = list(h0_sched)
        assert len(H0s) == NBLK

    nc = bass.Bass()
    lhs = nc.dram_tensor("lhs", [KCH, 128, PSL], f16, kind="ExternalInput")
    rhs = nc.dram_tensor("rhs", [KCH, 128, P], f16, kind="ExternalInput")
    ident = nc.dram_tensor("ident", [PBLK, PBLK], f32, kind="ExternalInput")
    out = nc.dram_tensor("out", [PBLK, 2 * NBLK], f32, kind="ExternalOutput")


    with TileContext(nc) as tc:
        with (
            tc.tile_pool(name="singles", bufs=1) as singles,
            tc.tile_pool(name="stage", bufs=stg_bufs) as stage,
            tc.tile_pool(name="wpool", bufs=w_bufs) as wpool,
            tc.tile_pool(name="stats", bufs=4 * NBLK) as stats,
            tc.tile_pool(name="ps", bufs=ps_bufs, space="PSUM") as ps,
        ):
            # ---- one-time loads ----
            # lhs + ident on the SWDGE queue, rhs chunks on HWDGE: the two
            # descriptor paths run in parallel so the first matmul starts
            # as soon as lhs + the first rhs chunk land
            lhs_sb = singles.tile([128, KCH, PSL], f16)
            for kc in range(KCH):
                nc.sync.dma_start(out=lhs_sb[:, kc, :], in_=lhs[kc, :, :])
            # rhs in separate per-chunk tiles so block-0 matmuls only wait on
            # their own chunk's DMA (single big DMA costs ~7us of startup)
            rhs_sb = []   # list of (q_start, q_end, tile)
            q = 0
            for j, rch in enumerate(rch_plan):
                rc = singles.tile([128, KCH, rch], f16, tag=f"rhs{j}")
                nc.sync.dma_start(
                    out=rc[:, :, :],
                    in_=rhs[:, :, q:q + rch].rearrange("k p n -> p k n"),
                )
                rhs_sb.append((q, q + rch, rc))
                q += rch
            assert q == P

            def rhs_at(q0):
                for qs, qe, rc in rhs_sb:
                    if qs <= q0 < qe:
                        assert q0 + FD <= qe
                        return rc, q0 - qs
                raise AssertionError(q0)
            ident_sb = singles.tile([PBLK, PBLK], f32)
            nc.gpsimd.dma_start(out=ident_sb, in_=ident[:, :])
            out_sb = singles.tile([PBLK, 2 * NBLK], f32)

            # per-block state carried across the software pipeline
            blk = [dict() for _ in range(NBLK)]

            def matmul_tile(pt, pb, q0):
                # pt[:, 0:QT] = lhs[:, pb-block].T @ rhs[:, q0:q0+QT]
                for kc in range(KCH):
                    for f in range(QT // FD):
                        rc, off = rhs_at(q0 + f * FD)
                        nc.tensor.matmul(
                            out=pt[:, f * FD:(f + 1) * FD],
                            lhsT=lhs_sb[:, kc, pb * PBLK:(pb + 1) * PBLK],
                            rhs=rc[:, kc, off:off + FD],
                            start=(kc == 0),
                            stop=(kc == KCH - 1),
                        )

            def pass1(pb):
                st = blk[pb]
                h0 = H0s[pb]
                n_q0, n_q1 = h0 // QT, (P - h0) // QT
                stg = stage.tile([128, h0], f32, tag="stg")
                nmx = (n_q0 if max_from_psum else 1) + n_q1
                mxp = stats.tile([128, nmx], f32, tag="mxp")
                # staged columns: matmul -> ACT copy to SBUF
                for j in range(n_q0):
                    pt = ps.tile([128, QT], f32, tag="pst")
                    matmul_tile(pt, pb, j * QT)
                    if max_from_psum:
                        nc.vector.reduce_max(out=mxp[:, j:j + 1], in_=pt, axis=X)
                    nc.scalar.activation(
                        out=stg[:, j * QT:(j + 1) * QT], in_=pt, func=Act.Copy,
                    )
                # recomputed columns: matmul -> DVE max only
                joff = n_q0 if max_from_psum else 1
                for j in range(n_q1):
                    pt = ps.tile([128, QT], f32, tag="pst")
                    matmul_tile(pt, pb, h0 + j * QT)
                    nc.vector.reduce_max(out=mxp[:, joff + j:joff + j + 1], in_=pt, axis=X)
                if not max_from_psum:
                    nc.vector.reduce_max(out=mxp[:, 0:1], in_=stg, axis=X)
                mx = stats.tile([128, 1], f32, tag="mx")
                nc.vector.reduce_max(out=mx, in_=mxp[:, 0:nmx], axis=X)
                # denom = (1-mx)/2 + eps ; s = 5/denom ; b = 10 - s
                denom = stats.tile([128, 1], f32, tag="denom")
                nc.vector.tensor_scalar(
                    out=denom, in0=mx, scalar1=-0.5, scalar2=0.5 + EPS,
                    op0=Alu.mult, op1=Alu.add,
                )
                rec = stats.tile([128, 1], f32, tag="rec")
                nc.vector.reciprocal(out=rec, in_=denom)
                b_ap = stats.tile([128, 1], f32, tag="b_ap")
                nc.vector.tensor_scalar(
                    out=b_ap, in0=rec, scalar1=-5.0, scalar2=10.0,
                    op0=Alu.mult, op1=Alu.add,
                )
                s_ap = stats.tile([128, 1], f32, tag="s_ap")
                nc.vector.tensor_scalar(
                    out=s_ap, in0=b_ap, scalar1=-1.0, scalar2=10.0,
                    op0=Alu.mult, op1=Alu.add,
                )
                st["stg"], st["s"], st["b"] = stg, s_ap, b_ap

            def pass2(pb):
                st = blk[pb]
                h0 = H0s[pb]
                n_q1 = (P - h0) // QT
                stg, s_ap, b_ap = st["stg"], st["s"], st["b"]
                sp = stats.tile([128, 1 + n_q1], f32, tag="sp")
                w0 = wpool.tile([128, h0], f32, tag="w0")
                nc.scalar.activation(
                    out=w0, in_=stg, func=Act.Exp,
                    bias=b_ap, scale=s_ap, accum_out=sp[:, 0:1],
                )
                for j in range(n_q1):
                    pt = ps.tile([128, QT], f32, tag="pst")
                    matmul_tile(pt, pb, h0 + j * QT)
                    w1 = wpool.tile([128, QT], f32, tag="w1")
                    nc.scalar.activation(
                        out=w1, in_=pt, func=Act.Exp,
                        bias=b_ap, scale=s_ap, accum_out=sp[:, 1 + j:2 + j],
                    )
                # diagonal band: w0[:, pb*128:(pb+1)*128] . ident, row-reduced
                band = wpool.tile([PBLK, PBLK], f32, tag="band")
                nc.vector.tensor_tensor(
                    out=band,
                    in0=w0[:, pb * PBLK:(pb + 1) * PBLK],
                    in1=ident_sb,
                    op=Alu.mult,
                )
                nc.vector.tensor_reduce(
                    out=out_sb[:, pb:pb + 1], in_=band, axis=X, op=Alu.add,
                )
                nc.vector.tensor_reduce(
                    out=out_sb[:, NBLK + pb:NBLK + pb + 1], in_=sp[:, 0:1 + n_q1],
                    axis=X, op=Alu.add,
                )

            # software pipeline: pass2 lags pass1 by `lag` blocks
            # (repeats>1 unrolls the whole loop for steady-state timing runs)
            for _ in range(repeats):
                for b in range(NBLK + lag):
                    if b < NBLK:
                        pass1(b)
                    if b >= lag:
                        pass2(b - lag)

            nc.sync.dma_start(out=out[:, :], in_=out_sb)

    _legalize_waits(nc)
    return nc


def _prep_inputs(I_features, T_features):
    """Host-side feature normalization (fp64) + per-core sharding (fp16)."""
    I = np.asarray(I_features, dtype=np.float64)
    T = np.asarray(T_features, dtype=np.float64)
    meanT = T.mean(axis=(0, 2, 3), keepdims=True)
    Ic = I - meanT
    Tc = T - meanT
    In = Ic / np.sqrt((Ic * Ic).sum(axis=1, keepdims=True))
    Tn = Tc / np.sqrt((Tc * Tc).sum(axis=1, keepdims=True))
    Iv = In.reshape(N_SAMP, C, P).astype(np.float16)
    Tv = Tn.reshape(N_SAMP, C, P).astype(np.float16)

    ident = np.eye(PBLK, dtype=np.float32)
    in_maps = []
    for c in range(N_CORES):
        n = c // CORES_PER_SAMPLE
        sl = (c % CORES_PER_SAMPLE) * PSL
        lhs = Iv[n][:, sl:sl + PSL].reshape(KCH, 128, PSL)
        rhs = np.roll(Tv[n], -sl, axis=1).reshape(KCH, 128, P)
        in_maps.append({
            "lhs": np.ascontiguousarray(lhs),
            "rhs": np.ascontiguousarray(rhs),
            "ident": ident,
        })
    return in_maps


def _combine(results):
    """Host-side reduction of per-core (diag, S) partials to the loss."""
    losses = []
    for n in range(N_SAMP):
        ratios = []
        for cs in range(CORES_PER_SAMPLE):
            r = results[n * CORES_PER_SAMPLE + cs]["out"].astype(np.float64)
            diag = r[:, 0:NBLK]     # [128, 8]: q = sl + pb*128 + i
            ssum = r[:, NBLK:]
            ratios.append(diag / ssum)
        m = 0.5 + 0.5 * np.mean(ratios)
        losses.append(-np.log(m))
    return np.float32(np.mean(losses))


def kernel(I_features, T_features, _trace=False):
    from concourse.bass_utils import run_bass_kernel_spmd

    if "nc" not in _CACHE:
        _CACHE["nc"] = _build_nc()
    nc = _CACHE["nc"]

    in_maps = _prep_inputs(I_features, T_features)
    res = run_bass_kernel_spmd(
        nc, in_maps, core_ids=list(range(N_CORES)), trace=_trace,
    )
    if _trace:
        _CACHE["last_result"] = res
    return _combine(res.results)
